# revision 18
# baseline (speedup 1.0000x reference)
"""Trainium2 Bass kernel for nn_Encoder_Postnet (length-regulator gather + per-frame linears).

Contract: kernel(**inputs) takes FULL numpy inputs (as produced by
setup_inputs) and returns the FULL [B, T, H] float32 output. Internally the
batch dim is sharded across 8 NeuronCores (pure data parallel, 4 batches per
core); the tiny Linear(1,H) params are replicated.

Per-core algorithm (BPC=4 batches, T=4096 frames, P=512 phonemes, H=512):
  1. idx[b,t] = cumsum_t(align[b,t] != align[b,t-1])  -- DVE compare + scan
  2. PE-transpose idx chunks into per-partition layout -> gather offsets
  3. grouped indirect-DMA gathers: enc[b, idx[b,t], :] rows from HBM
  4. rank-4 PE matmul per 128-frame tile:
       [pitch_t, beats_t, t, 1] @ [w_pitch; w_beats; w_pos; b_sum]
  5. one DVE add (gathered + psum) per tile, HWDGE write out
"""

import sys

if "/opt/trn_rl_repo" not in sys.path:
    sys.path.insert(0, "/opt/trn_rl_repo")

from contextlib import ExitStack

import numpy as np

import concourse.bass as bass
import concourse.tile as tile
from concourse import bacc, mybir
from concourse.bass_utils import run_bass_kernel_spmd
from concourse.masks import make_identity

B, T, P, H = 32, 4096, 512, 512
NCORES = 8
BPC = B // NCORES            # batches per core
TILE_T = 128                 # frames per tile (partition dim)
NCHUNK = T // TILE_T         # 32 tiles per batch
K_MM = 11                    # bf16 matmul contraction (hi/lo split, see below)
F32 = mybir.dt.float32
BF16 = mybir.dt.bfloat16
I32 = mybir.dt.int32
ADD = mybir.AluOpType.add
SUB = mybir.AluOpType.subtract
NE = mybir.AluOpType.not_equal


def _emit(ctx: ExitStack, tc: tile.TileContext, enc, pitch, beats, align,
          w_pitch, w_beats, w_pos, b_pitch, b_beats, b_pos, aux, out):
    nc = tc.nc
    const = ctx.enter_context(tc.tile_pool(name="const", bufs=1))
    apool = ctx.enter_context(tc.tile_pool(name="apool", bufs=2))
    gpool = ctx.enter_context(tc.tile_pool(name="gpool", bufs=6))
    ppool = ctx.enter_context(tc.tile_pool(name="ppool", bufs=4, space="PSUM"))
    tpsum = ctx.enter_context(tc.tile_pool(name="tpsum", bufs=1, space="PSUM"))

    # --- W [11, H] bf16. fp32 matmul lowers to two ~1us passes on TRN2, so
    # the rank-update runs in bf16 with hi/lo-split weights for precision:
    #   pos*w_pos = (t_hi + t_lo) * (w_hi + w_lo),  t_hi = 16*(t//16), exact
    # W rows: [wpos_hi, wpos_lo, wpos_hi, wpos_lo, wpit_hi, wpit_lo,
    #          wbea_hi, wbea_lo, b_pitch, b_beats, b_pos]
    # A rows: [t_hi, t_hi, t_lo, t_lo, pitch, pitch, beats, beats, 1, 1, 1]
    # All row writes land on non-zero partitions via DMA only (compute ops
    # require start partition in {0,32,64,96}).
    W = const.tile([K_MM, H], BF16)
    for i, w in enumerate([w_pos, w_pitch, w_beats]):
        wsrc = const.tile([1, H], F32, tag=f"wsrc{i}")
        nc.sync.dma_start(wsrc[:], w[None, :])
        whi = const.tile([1, H], BF16, tag=f"whi{i}")
        nc.vector.tensor_copy(whi[:], wsrc[:])
        wlo = const.tile([1, H], BF16, tag=f"wlo{i}")
        nc.vector.tensor_tensor(wlo[:], wsrc[:], whi[:], op=SUB)
        if i == 0:  # w_pos used by both t_hi and t_lo rows
            nc.sync.dma_start(W[0:1, :], whi[:])
            nc.sync.dma_start(W[1:2, :], wlo[:])
            nc.sync.dma_start(W[2:3, :], whi[:])
            nc.sync.dma_start(W[3:4, :], wlo[:])
        else:
            nc.sync.dma_start(W[2 + 2 * i:3 + 2 * i, :], whi[:])
            nc.sync.dma_start(W[3 + 2 * i:4 + 2 * i, :], wlo[:])
    nc.gpsimd.dma_start(W[8:9, :], b_pitch[None, :])   # f32 -> bf16 cast DMA
    nc.gpsimd.dma_start(W[9:10, :], b_beats[None, :])
    nc.gpsimd.dma_start(W[10:11, :], b_pos[None, :])

    # --- idx[b, t] = cumsum_t(align[b, t] != align[b, t-1]), in f32 (exact)
    align_sb = const.tile([BPC, T], I32)
    nc.sync.dma_start(align_sb[:], align[:])
    change = const.tile([BPC, T], F32)
    nc.vector.memset(change[:, 0:1], 0.0)
    nc.vector.tensor_tensor(change[:, 1:T], align_sb[:, 1:T],
                            align_sb[:, 0:T - 1], op=NE)
    zeros = const.tile([BPC, T], F32)
    nc.vector.memset(zeros[:], 0.0)
    idxf = const.tile([BPC, T], F32)
    nc.vector.tensor_tensor_scan(idxf[:], change[:], zeros[:], 0.0,
                                 op0=ADD, op1=ADD)

    # --- transpose to per-partition layout: idxT[p, c*BPC + b] = idx[b, c*128+p]
    ident = const.tile([BPC, BPC], F32)
    make_identity(nc, ident[:])
    idxT_ps = tpsum.tile([TILE_T, NCHUNK * BPC], F32)
    for c in range(NCHUNK):
        nc.tensor.transpose(idxT_ps[:, c * BPC:(c + 1) * BPC],
                            idxf[:, c * TILE_T:(c + 1) * TILE_T], ident[:])
    idxT = const.tile([TILE_T, NCHUNK * BPC], F32)
    nc.vector.tensor_copy(idxT[:], idxT_ps[:])

    # --- per-batch row offsets into enc viewed as [(BPC*P), H]: idx + b*P
    idxT3 = idxT[:].rearrange("p (c b) -> p b c", b=BPC)  # [128, BPC, NCHUNK]
    offs = []
    for b in range(BPC):
        ob = const.tile([TILE_T, NCHUNK], I32, tag=f"offs{b}")
        nc.vector.tensor_scalar_add(ob[:], idxT3[:, b, :], float(b * P))
        offs.append(ob)

    # --- main loop
    for b in range(BPC):
        # A [11, T] bf16: [t_hi, t_hi, t_lo, t_lo, pitch, pitch, beats,
        # beats, 1, 1, 1]; t_hi/t_lo/ones from host aux (exact in bf16),
        # pitch/beats cast f32->bf16 during the SWDGE DMA
        A = apool.tile([K_MM, T], BF16)
        nc.sync.dma_start(A[0:4, :], aux[0:4, :])
        nc.gpsimd.dma_start(A[4:5, :], pitch[b:b + 1, :])
        nc.gpsimd.dma_start(A[5:6, :], pitch[b:b + 1, :])
        nc.gpsimd.dma_start(A[6:7, :], beats[b:b + 1, :])
        nc.gpsimd.dma_start(A[7:8, :], beats[b:b + 1, :])
        nc.sync.dma_start(A[8:11, :], aux[4:7, :])

        for c in range(NCHUNK):
            # HW indirect DMA consumes exactly one offset per dest partition,
            # so gathers are per-chunk: 128 descriptors x one H-row each
            gt = gpool.tile([TILE_T, H], F32)
            nc.gpsimd.indirect_dma_start(
                out=gt[:],
                out_offset=None,
                in_=enc[:],
                in_offset=bass.IndirectOffsetOnAxis(
                    ap=offs[b][:, c:c + 1], axis=0),
            )
            ps = ppool.tile([TILE_T, H], F32)
            nc.tensor.matmul(ps[:],
                             lhsT=A[:, c * TILE_T:(c + 1) * TILE_T],
                             rhs=W[:], start=True, stop=True)
            nc.vector.tensor_tensor(gt[:], gt[:], ps[:], op=ADD)
            nc.sync.dma_start(
                out[b * T + c * TILE_T: b * T + (c + 1) * TILE_T, :], gt[:])


_CACHED = None


def _build():
    global _CACHED
    if _CACHED is not None:
        return _CACHED
    nc = bacc.Bacc("TRN2", target_bir_lowering=False, debug=False)
    enc = nc.dram_tensor("enc", (BPC * P, H), F32, kind="ExternalInput").ap()
    pitch = nc.dram_tensor("pitch", (BPC, T), F32, kind="ExternalInput").ap()
    beats = nc.dram_tensor("beats", (BPC, T), F32, kind="ExternalInput").ap()
    align = nc.dram_tensor("align", (BPC, T), I32, kind="ExternalInput").ap()
    w_pitch = nc.dram_tensor("w_pitch", (H,), F32, kind="ExternalInput").ap()
    w_beats = nc.dram_tensor("w_beats", (H,), F32, kind="ExternalInput").ap()
    w_pos = nc.dram_tensor("w_pos", (H,), F32, kind="ExternalInput").ap()
    b_pitch = nc.dram_tensor("b_pitch", (H,), F32, kind="ExternalInput").ap()
    b_beats = nc.dram_tensor("b_beats", (H,), F32, kind="ExternalInput").ap()
    b_pos = nc.dram_tensor("b_pos", (H,), F32, kind="ExternalInput").ap()
    aux = nc.dram_tensor("aux", (7, T), BF16, kind="ExternalInput").ap()
    out = nc.dram_tensor("out", (BPC * T, H), F32, kind="ExternalOutput").ap()

    with tile.TileContext(nc) as tc:
        with ExitStack() as ctx:
            _emit(ctx, tc, enc, pitch, beats, align, w_pitch, w_beats, w_pos,
                  b_pitch, b_beats, b_pos, aux, out)
    nc.compile()
    _CACHED = nc
    return nc


def make_in_maps(encoder_out, pitch, beats, align_phone,
                 w_pitch, b_pitch, w_beats, b_beats, w_pos, b_pos):
    import ml_dtypes
    t = np.arange(T, dtype=np.float32)
    t_hi = np.float32(16.0) * np.floor(t / 16.0).astype(np.float32)
    t_lo = t - t_hi
    ones = np.ones(T, np.float32)
    aux = np.stack([t_hi, t_hi, t_lo, t_lo, ones, ones, ones]).astype(
        ml_dtypes.bfloat16)
    reps = {
        "aux": aux,
        "w_pitch": np.ascontiguousarray(w_pitch, np.float32),
        "w_beats": np.ascontiguousarray(w_beats, np.float32),
        "w_pos": np.ascontiguousarray(w_pos, np.float32),
        "b_pitch": np.ascontiguousarray(b_pitch, np.float32),
        "b_beats": np.ascontiguousarray(b_beats, np.float32),
        "b_pos": np.ascontiguousarray(b_pos, np.float32),
    }
    in_maps = []
    for r in range(NCORES):
        s = slice(r * BPC, (r + 1) * BPC)
        in_maps.append({
            "enc": np.ascontiguousarray(
                encoder_out[s], np.float32).reshape(BPC * P, H),
            "pitch": np.ascontiguousarray(pitch[s], np.float32),
            "beats": np.ascontiguousarray(beats[s], np.float32),
            "align": np.ascontiguousarray(align_phone[s], np.int32),
            **reps,
        })
    return in_maps


def kernel(encoder_out, pitch, beats, w_pitch, b_pitch, w_beats, b_beats,
           w_pos, b_pos, align_phone, _trace=False):
    nc = _build()
    in_maps = make_in_maps(encoder_out, pitch, beats, align_phone,
                           w_pitch, b_pitch, w_beats, b_beats, w_pos, b_pos)
    res = run_bass_kernel_spmd(nc, in_maps, core_ids=list(range(NCORES)),
                               trace=_trace)
    out = np.concatenate(
        [res.results[r]["out"].reshape(BPC, T, H) for r in range(NCORES)],
        axis=0)
    if _trace:
        kernel.last_results = res
    return out


# revision 21
# speedup vs baseline: 1.1469x; 1.1469x over previous
"""Trainium2 Bass kernel for nn_Encoder_Postnet (length-regulator gather + per-frame linears).

Contract: kernel(**inputs) takes FULL numpy inputs (as produced by
setup_inputs) and returns the FULL [B, T, H] float32 output. Internally the
batch dim is sharded across 8 NeuronCores (pure data parallel, 4 batches per
core); the tiny Linear(1,H) params are replicated.

Per-core algorithm (BPC=4 batches, T=4096 frames, P=512 phonemes, H=512):
  1. idx[b,t] = cumsum_t(align[b,t] != align[b,t-1])  -- DVE compare + scan
  2. PE-transpose idx chunks into per-partition layout -> gather offsets
  3. grouped indirect-DMA gathers: enc[b, idx[b,t], :] rows from HBM
  4. rank-4 PE matmul per 128-frame tile:
       [pitch_t, beats_t, t, 1] @ [w_pitch; w_beats; w_pos; b_sum]
  5. one DVE add (gathered + psum) per tile, HWDGE write out
"""

import sys

if "/opt/trn_rl_repo" not in sys.path:
    sys.path.insert(0, "/opt/trn_rl_repo")

from contextlib import ExitStack

import numpy as np

import concourse.bass as bass
import concourse.tile as tile
from concourse import bacc, mybir
from concourse.bass_utils import run_bass_kernel_spmd
from concourse.masks import make_identity

B, T, P, H = 32, 4096, 512, 512
NCORES = 8
BPC = B // NCORES            # batches per core
TILE_T = 128                 # frames per tile (partition dim)
NCHUNK = T // TILE_T         # 32 tiles per batch
K_MM = 11                    # bf16 matmul contraction (hi/lo split, see below)
F32 = mybir.dt.float32
BF16 = mybir.dt.bfloat16
I32 = mybir.dt.int32
ADD = mybir.AluOpType.add
SUB = mybir.AluOpType.subtract
NE = mybir.AluOpType.not_equal


def _emit(ctx: ExitStack, tc: tile.TileContext, enc, pitch, beats, align,
          w_pitch, w_beats, w_pos, b_pitch, b_beats, b_pos, aux, out):
    nc = tc.nc
    const = ctx.enter_context(tc.tile_pool(name="const", bufs=1))
    apool = ctx.enter_context(tc.tile_pool(name="apool", bufs=2))
    gpool = ctx.enter_context(tc.tile_pool(name="gpool", bufs=10))
    ppool = ctx.enter_context(tc.tile_pool(name="ppool", bufs=6, space="PSUM"))
    tpsum = ctx.enter_context(tc.tile_pool(name="tpsum", bufs=1, space="PSUM"))

    # --- W [11, H] bf16. fp32 matmul lowers to two ~1us passes on TRN2, so
    # the rank-update runs in bf16 with hi/lo-split weights for precision:
    #   pos*w_pos = (t_hi + t_lo) * (w_hi + w_lo),  t_hi = 16*(t//16), exact
    # W rows: [wpos_hi, wpos_lo, wpos_hi, wpos_lo, wpit_hi, wpit_lo,
    #          wbea_hi, wbea_lo, b_pitch, b_beats, b_pos]
    # A rows: [t_hi, t_hi, t_lo, t_lo, pitch, pitch, beats, beats, 1, 1, 1]
    # All row writes land on non-zero partitions via DMA only (compute ops
    # require start partition in {0,32,64,96}).
    W = const.tile([K_MM, H], BF16)
    for i, w in enumerate([w_pos, w_pitch, w_beats]):
        wsrc = const.tile([1, H], F32, tag=f"wsrc{i}")
        nc.sync.dma_start(wsrc[:], w[None, :])
        whi = const.tile([1, H], BF16, tag=f"whi{i}")
        nc.vector.tensor_copy(whi[:], wsrc[:])
        wlo = const.tile([1, H], BF16, tag=f"wlo{i}")
        nc.vector.tensor_tensor(wlo[:], wsrc[:], whi[:], op=SUB)
        if i == 0:  # w_pos used by both t_hi and t_lo rows
            nc.sync.dma_start(W[0:1, :], whi[:])
            nc.sync.dma_start(W[1:2, :], wlo[:])
            nc.sync.dma_start(W[2:3, :], whi[:])
            nc.sync.dma_start(W[3:4, :], wlo[:])
        else:
            nc.sync.dma_start(W[2 + 2 * i:3 + 2 * i, :], whi[:])
            nc.sync.dma_start(W[3 + 2 * i:4 + 2 * i, :], wlo[:])
    nc.gpsimd.dma_start(W[8:9, :], b_pitch[None, :])   # f32 -> bf16 cast DMA
    nc.gpsimd.dma_start(W[9:10, :], b_beats[None, :])
    nc.gpsimd.dma_start(W[10:11, :], b_pos[None, :])

    # --- idx[b, t] = cumsum_t(align[b, t] != align[b, t-1]), in f32 (exact)
    align_sb = const.tile([BPC, T], I32)
    nc.sync.dma_start(align_sb[:], align[:])
    change = const.tile([BPC, T], F32)
    nc.vector.memset(change[:, 0:1], 0.0)
    nc.vector.tensor_tensor(change[:, 1:T], align_sb[:, 1:T],
                            align_sb[:, 0:T - 1], op=NE)
    zeros = const.tile([BPC, T], F32)
    nc.vector.memset(zeros[:], 0.0)
    idxf = const.tile([BPC, T], F32)
    nc.vector.tensor_tensor_scan(idxf[:], change[:], zeros[:], 0.0,
                                 op0=ADD, op1=ADD)

    # --- transpose to per-partition layout: idxT[p, c*BPC + b] = idx[b, c*128+p]
    ident = const.tile([BPC, BPC], F32)
    make_identity(nc, ident[:])
    idxT_ps = tpsum.tile([TILE_T, NCHUNK * BPC], F32)
    for c in range(NCHUNK):
        nc.tensor.transpose(idxT_ps[:, c * BPC:(c + 1) * BPC],
                            idxf[:, c * TILE_T:(c + 1) * TILE_T], ident[:])
    idxT = const.tile([TILE_T, NCHUNK * BPC], F32)
    nc.vector.tensor_copy(idxT[:], idxT_ps[:])

    # --- per-batch row offsets into enc viewed as [(BPC*P), H]: idx + b*P
    idxT3 = idxT[:].rearrange("p (c b) -> p b c", b=BPC)  # [128, BPC, NCHUNK]
    offs = []
    for b in range(BPC):
        ob = const.tile([TILE_T, NCHUNK], I32, tag=f"offs{b}")
        nc.vector.tensor_scalar_add(ob[:], idxT3[:, b, :], float(b * P))
        offs.append(ob)

    # --- main loop
    for b in range(BPC):
        # A [11, T] bf16: [t_hi, t_hi, t_lo, t_lo, pitch, pitch, beats,
        # beats, 1, 1, 1]; t_hi/t_lo/ones from host aux (exact in bf16),
        # pitch/beats cast f32->bf16 during the SWDGE DMA
        A = apool.tile([K_MM, T], BF16)
        nc.sync.dma_start(A[0:4, :], aux[0:4, :])
        nc.gpsimd.dma_start(A[4:5, :], pitch[b:b + 1, :])
        nc.gpsimd.dma_start(A[5:6, :], pitch[b:b + 1, :])
        nc.gpsimd.dma_start(A[6:7, :], beats[b:b + 1, :])
        nc.gpsimd.dma_start(A[7:8, :], beats[b:b + 1, :])
        nc.sync.dma_start(A[8:11, :], aux[4:7, :])

        for c in range(NCHUNK):
            # HW indirect DMA consumes exactly one offset per dest partition,
            # so gathers are per-chunk: 128 descriptors x one H-row each
            gt = gpool.tile([TILE_T, H], F32)
            nc.gpsimd.indirect_dma_start(
                out=gt[:],
                out_offset=None,
                in_=enc[:],
                in_offset=bass.IndirectOffsetOnAxis(
                    ap=offs[b][:, c:c + 1], axis=0),
            )
            ps = ppool.tile([TILE_T, H], F32)
            nc.tensor.matmul(ps[:],
                             lhsT=A[:, c * TILE_T:(c + 1) * TILE_T],
                             rhs=W[:], start=True, stop=True)
            nc.vector.tensor_tensor(gt[:], gt[:], ps[:], op=ADD)
            # alternate the two HWDGE rings (SP via sync, ACT via scalar)
            weng = nc.sync if c % 2 == 0 else nc.scalar
            weng.dma_start(
                out[b * T + c * TILE_T: b * T + (c + 1) * TILE_T, :], gt[:])


_CACHED = None


def _build():
    global _CACHED
    if _CACHED is not None:
        return _CACHED
    nc = bacc.Bacc("TRN2", target_bir_lowering=False, debug=False,
                   num_swdge_queues=2)
    enc = nc.dram_tensor("enc", (BPC * P, H), F32, kind="ExternalInput").ap()
    pitch = nc.dram_tensor("pitch", (BPC, T), F32, kind="ExternalInput").ap()
    beats = nc.dram_tensor("beats", (BPC, T), F32, kind="ExternalInput").ap()
    align = nc.dram_tensor("align", (BPC, T), I32, kind="ExternalInput").ap()
    w_pitch = nc.dram_tensor("w_pitch", (H,), F32, kind="ExternalInput").ap()
    w_beats = nc.dram_tensor("w_beats", (H,), F32, kind="ExternalInput").ap()
    w_pos = nc.dram_tensor("w_pos", (H,), F32, kind="ExternalInput").ap()
    b_pitch = nc.dram_tensor("b_pitch", (H,), F32, kind="ExternalInput").ap()
    b_beats = nc.dram_tensor("b_beats", (H,), F32, kind="ExternalInput").ap()
    b_pos = nc.dram_tensor("b_pos", (H,), F32, kind="ExternalInput").ap()
    aux = nc.dram_tensor("aux", (7, T), BF16, kind="ExternalInput").ap()
    out = nc.dram_tensor("out", (BPC * T, H), F32, kind="ExternalOutput").ap()

    with tile.TileContext(nc) as tc:
        with ExitStack() as ctx:
            _emit(ctx, tc, enc, pitch, beats, align, w_pitch, w_beats, w_pos,
                  b_pitch, b_beats, b_pos, aux, out)
    nc.compile()
    _CACHED = nc
    return nc


def make_in_maps(encoder_out, pitch, beats, align_phone,
                 w_pitch, b_pitch, w_beats, b_beats, w_pos, b_pos):
    import ml_dtypes
    t = np.arange(T, dtype=np.float32)
    t_hi = np.float32(16.0) * np.floor(t / 16.0).astype(np.float32)
    t_lo = t - t_hi
    ones = np.ones(T, np.float32)
    aux = np.stack([t_hi, t_hi, t_lo, t_lo, ones, ones, ones]).astype(
        ml_dtypes.bfloat16)
    reps = {
        "aux": aux,
        "w_pitch": np.ascontiguousarray(w_pitch, np.float32),
        "w_beats": np.ascontiguousarray(w_beats, np.float32),
        "w_pos": np.ascontiguousarray(w_pos, np.float32),
        "b_pitch": np.ascontiguousarray(b_pitch, np.float32),
        "b_beats": np.ascontiguousarray(b_beats, np.float32),
        "b_pos": np.ascontiguousarray(b_pos, np.float32),
    }
    in_maps = []
    for r in range(NCORES):
        s = slice(r * BPC, (r + 1) * BPC)
        in_maps.append({
            "enc": np.ascontiguousarray(
                encoder_out[s], np.float32).reshape(BPC * P, H),
            "pitch": np.ascontiguousarray(pitch[s], np.float32),
            "beats": np.ascontiguousarray(beats[s], np.float32),
            "align": np.ascontiguousarray(align_phone[s], np.int32),
            **reps,
        })
    return in_maps


def kernel(encoder_out, pitch, beats, w_pitch, b_pitch, w_beats, b_beats,
           w_pos, b_pos, align_phone, _trace=False):
    nc = _build()
    in_maps = make_in_maps(encoder_out, pitch, beats, align_phone,
                           w_pitch, b_pitch, w_beats, b_beats, w_pos, b_pos)
    res = run_bass_kernel_spmd(nc, in_maps, core_ids=list(range(NCORES)),
                               trace=_trace)
    out = np.concatenate(
        [res.results[r]["out"].reshape(BPC, T, H) for r in range(NCORES)],
        axis=0)
    if _trace:
        kernel.last_results = res
    return out


# revision 24
# speedup vs baseline: 1.2124x; 1.0571x over previous
"""Trainium2 Bass kernel for nn_Encoder_Postnet (length-regulator gather + per-frame linears).

Contract: kernel(**inputs) takes FULL numpy inputs (as produced by
setup_inputs) and returns the FULL [B, T, H] float32 output. Internally the
batch dim is sharded across 8 NeuronCores (pure data parallel, 4 batches per
core); the tiny Linear(1,H) params are replicated.

Per-core algorithm (BPC=4 batches, T=4096 frames, P=512 phonemes, H=512):
  1. idx[b,t] = cumsum_t(align[b,t] != align[b,t-1])  -- DVE compare + scan
  2. PE-transpose idx chunks into per-partition layout -> gather offsets
  3. grouped indirect-DMA gathers: enc[b, idx[b,t], :] rows from HBM
  4. rank-4 PE matmul per 128-frame tile:
       [pitch_t, beats_t, t, 1] @ [w_pitch; w_beats; w_pos; b_sum]
  5. one DVE add (gathered + psum) per tile, HWDGE write out
"""

import sys

if "/opt/trn_rl_repo" not in sys.path:
    sys.path.insert(0, "/opt/trn_rl_repo")

from contextlib import ExitStack

import numpy as np

import concourse.bass as bass
import concourse.tile as tile
from concourse import bacc, mybir
from concourse.bass_utils import run_bass_kernel_spmd
from concourse.masks import make_identity

B, T, P, H = 32, 4096, 512, 512
NCORES = 8
BPC = B // NCORES            # batches per core
TILE_T = 128                 # frames per tile (partition dim)
NCHUNK = T // TILE_T         # 32 tiles per batch
K_MM = 11                    # bf16 matmul contraction (hi/lo split, see below)
F32 = mybir.dt.float32
BF16 = mybir.dt.bfloat16
I32 = mybir.dt.int32
ADD = mybir.AluOpType.add
SUB = mybir.AluOpType.subtract
NE = mybir.AluOpType.not_equal


def _emit(ctx: ExitStack, tc: tile.TileContext, enc, pitch, beats, align,
          w_pitch, w_beats, w_pos, b_pitch, b_beats, b_pos, aux, out):
    nc = tc.nc
    const = ctx.enter_context(tc.tile_pool(name="const", bufs=1))
    apool = ctx.enter_context(tc.tile_pool(name="apool", bufs=1))
    gpool = ctx.enter_context(tc.tile_pool(name="gpool", bufs=10))
    ppool = ctx.enter_context(tc.tile_pool(name="ppool", bufs=6, space="PSUM"))
    tpsum = ctx.enter_context(tc.tile_pool(name="tpsum", bufs=1, space="PSUM"))

    # --- W [11, H] bf16. fp32 matmul lowers to two ~1us passes on TRN2, so
    # the rank-update runs in bf16 with hi/lo-split weights for precision:
    #   pos*w_pos = (t_hi + t_lo) * (w_hi + w_lo),  t_hi = 16*(t//16), exact
    # W rows: [wpos_hi, wpos_lo, wpos_hi, wpos_lo, wpit_hi, wpit_lo,
    #          wbea_hi, wbea_lo, b_pitch, b_beats, b_pos]
    # A rows: [t_hi, t_hi, t_lo, t_lo, pitch, pitch, beats, beats, 1, 1, 1]
    # All row writes land on non-zero partitions via DMA only (compute ops
    # require start partition in {0,32,64,96}).
    W = const.tile([K_MM, H], BF16)
    for i, w in enumerate([w_pos, w_pitch, w_beats]):
        wsrc = const.tile([1, H], F32, tag=f"wsrc{i}")
        nc.sync.dma_start(wsrc[:], w[None, :])
        whi = const.tile([1, H], BF16, tag=f"whi{i}")
        nc.vector.tensor_copy(whi[:], wsrc[:])
        wlo = const.tile([1, H], BF16, tag=f"wlo{i}")
        nc.vector.tensor_tensor(wlo[:], wsrc[:], whi[:], op=SUB)
        if i == 0:  # w_pos used by both t_hi and t_lo rows
            nc.sync.dma_start(W[0:1, :], whi[:])
            nc.sync.dma_start(W[1:2, :], wlo[:])
            nc.sync.dma_start(W[2:3, :], whi[:])
            nc.sync.dma_start(W[3:4, :], wlo[:])
        else:
            nc.sync.dma_start(W[2 + 2 * i:3 + 2 * i, :], whi[:])
            nc.sync.dma_start(W[3 + 2 * i:4 + 2 * i, :], wlo[:])
    nc.gpsimd.dma_start(W[8:9, :], b_pitch[None, :])   # f32 -> bf16 cast DMA
    nc.gpsimd.dma_start(W[9:10, :], b_beats[None, :])
    nc.gpsimd.dma_start(W[10:11, :], b_pos[None, :])

    # --- A tiles, persistent per batch (loaded up front, no idx dependency)
    # A [11, T] bf16: [t_hi, t_hi, t_lo, t_lo, pitch, pitch, beats,
    # beats, 1, 1, 1]; t_hi/t_lo/ones from host aux (exact in bf16),
    # pitch/beats cast f32->bf16 during the SWDGE DMA
    As = []
    for b in range(BPC):
        A = apool.tile([K_MM, T], BF16, tag=f"A{b}")
        nc.sync.dma_start(A[0:4, :], aux[0:4, :])
        nc.gpsimd.dma_start(A[4:5, :], pitch[b:b + 1, :])
        nc.gpsimd.dma_start(A[5:6, :], pitch[b:b + 1, :])
        nc.gpsimd.dma_start(A[6:7, :], beats[b:b + 1, :])
        nc.gpsimd.dma_start(A[7:8, :], beats[b:b + 1, :])
        nc.sync.dma_start(A[8:11, :], aux[4:7, :])
        As.append(A)

    # --- idx[b, t] = cumsum_t(align[b, t] != align[b, t-1]), computed in a
    # 4-stage pipeline (chained scan carries) so the first gathers start
    # after ~1/4 of the setup instead of after the whole chain
    align_sb = const.tile([BPC, T], I32)
    nc.sync.dma_start(align_sb[:], align[:])
    change = const.tile([BPC, T], F32)
    nc.vector.memset(change[:, 0:1], 0.0)
    zeros = const.tile([BPC, T], F32)
    nc.vector.memset(zeros[:], 0.0)
    idxf = const.tile([BPC, T], F32)
    ident = const.tile([BPC, BPC], F32)
    make_identity(nc, ident[:])
    idxT_ps = tpsum.tile([TILE_T, NCHUNK * BPC], F32)
    idxT = const.tile([TILE_T, NCHUNK * BPC], F32)
    offs = [const.tile([TILE_T, NCHUNK], I32, tag=f"offs{b}",
                       name=f"offs{b}")
            for b in range(BPC)]
    idxT3 = idxT[:].rearrange("p (c b) -> p b c", b=BPC)  # [128, BPC, NCHUNK]

    NSTAGE = 4
    CPS = NCHUNK // NSTAGE            # chunks per stage
    SW = CPS * TILE_T                 # scan window
    for st in range(NSTAGE):
        lo, hi = st * SW, (st + 1) * SW
        s0 = max(lo, 1)
        nc.vector.tensor_tensor(change[:, s0:hi], align_sb[:, s0:hi],
                                align_sb[:, s0 - 1:hi - 1], op=NE)
        carry = 0.0 if st == 0 else idxf[:, lo - 1:lo]
        nc.vector.tensor_tensor_scan(idxf[:, lo:hi], change[:, lo:hi],
                                     zeros[:, lo:hi], carry,
                                     op0=ADD, op1=ADD)
        for c in range(st * CPS, (st + 1) * CPS):
            nc.tensor.transpose(idxT_ps[:, c * BPC:(c + 1) * BPC],
                                idxf[:, c * TILE_T:(c + 1) * TILE_T],
                                ident[:])
        cl, ch = st * CPS * BPC, (st + 1) * CPS * BPC
        nc.vector.tensor_copy(idxT[:, cl:ch], idxT_ps[:, cl:ch])
        for b in range(BPC):
            nc.vector.tensor_scalar_add(
                offs[b][:, st * CPS:(st + 1) * CPS],
                idxT3[:, b, st * CPS:(st + 1) * CPS], float(b * P))

        # --- main loop for this stage's chunks, all batches
        for b in range(BPC):
            for c in range(st * CPS, (st + 1) * CPS):
                # HW indirect DMA consumes exactly one offset per dest
                # partition: per-chunk gathers, 128 descriptors x one H-row
                gt = gpool.tile([TILE_T, H], F32)
                nc.gpsimd.indirect_dma_start(
                    out=gt[:],
                    out_offset=None,
                    in_=enc[:],
                    in_offset=bass.IndirectOffsetOnAxis(
                        ap=offs[b][:, c:c + 1], axis=0),
                )
                ps = ppool.tile([TILE_T, H], F32)
                nc.tensor.matmul(ps[:],
                                 lhsT=As[b][:, c * TILE_T:(c + 1) * TILE_T],
                                 rhs=W[:], start=True, stop=True)
                nc.vector.tensor_tensor(gt[:], gt[:], ps[:], op=ADD)
                # alternate the two HWDGE rings (SP via sync, ACT via scalar)
                weng = nc.sync if c % 2 == 0 else nc.scalar
                weng.dma_start(
                    out[b * T + c * TILE_T: b * T + (c + 1) * TILE_T, :],
                    gt[:])


_CACHED = None


def _build():
    global _CACHED
    if _CACHED is not None:
        return _CACHED
    nc = bacc.Bacc("TRN2", target_bir_lowering=False, debug=False,
                   num_swdge_queues=2)
    enc = nc.dram_tensor("enc", (BPC * P, H), F32, kind="ExternalInput").ap()
    pitch = nc.dram_tensor("pitch", (BPC, T), F32, kind="ExternalInput").ap()
    beats = nc.dram_tensor("beats", (BPC, T), F32, kind="ExternalInput").ap()
    align = nc.dram_tensor("align", (BPC, T), I32, kind="ExternalInput").ap()
    w_pitch = nc.dram_tensor("w_pitch", (H,), F32, kind="ExternalInput").ap()
    w_beats = nc.dram_tensor("w_beats", (H,), F32, kind="ExternalInput").ap()
    w_pos = nc.dram_tensor("w_pos", (H,), F32, kind="ExternalInput").ap()
    b_pitch = nc.dram_tensor("b_pitch", (H,), F32, kind="ExternalInput").ap()
    b_beats = nc.dram_tensor("b_beats", (H,), F32, kind="ExternalInput").ap()
    b_pos = nc.dram_tensor("b_pos", (H,), F32, kind="ExternalInput").ap()
    aux = nc.dram_tensor("aux", (7, T), BF16, kind="ExternalInput").ap()
    out = nc.dram_tensor("out", (BPC * T, H), F32, kind="ExternalOutput").ap()

    with tile.TileContext(nc) as tc:
        with ExitStack() as ctx:
            _emit(ctx, tc, enc, pitch, beats, align, w_pitch, w_beats, w_pos,
                  b_pitch, b_beats, b_pos, aux, out)
    nc.compile()
    _CACHED = nc
    return nc


def make_in_maps(encoder_out, pitch, beats, align_phone,
                 w_pitch, b_pitch, w_beats, b_beats, w_pos, b_pos):
    import ml_dtypes
    t = np.arange(T, dtype=np.float32)
    t_hi = np.float32(16.0) * np.floor(t / 16.0).astype(np.float32)
    t_lo = t - t_hi
    ones = np.ones(T, np.float32)
    aux = np.stack([t_hi, t_hi, t_lo, t_lo, ones, ones, ones]).astype(
        ml_dtypes.bfloat16)
    reps = {
        "aux": aux,
        "w_pitch": np.ascontiguousarray(w_pitch, np.float32),
        "w_beats": np.ascontiguousarray(w_beats, np.float32),
        "w_pos": np.ascontiguousarray(w_pos, np.float32),
        "b_pitch": np.ascontiguousarray(b_pitch, np.float32),
        "b_beats": np.ascontiguousarray(b_beats, np.float32),
        "b_pos": np.ascontiguousarray(b_pos, np.float32),
    }
    in_maps = []
    for r in range(NCORES):
        s = slice(r * BPC, (r + 1) * BPC)
        in_maps.append({
            "enc": np.ascontiguousarray(
                encoder_out[s], np.float32).reshape(BPC * P, H),
            "pitch": np.ascontiguousarray(pitch[s], np.float32),
            "beats": np.ascontiguousarray(beats[s], np.float32),
            "align": np.ascontiguousarray(align_phone[s], np.int32),
            **reps,
        })
    return in_maps


def kernel(encoder_out, pitch, beats, w_pitch, b_pitch, w_beats, b_beats,
           w_pos, b_pos, align_phone, _trace=False):
    nc = _build()
    in_maps = make_in_maps(encoder_out, pitch, beats, align_phone,
                           w_pitch, b_pitch, w_beats, b_beats, w_pos, b_pos)
    res = run_bass_kernel_spmd(nc, in_maps, core_ids=list(range(NCORES)),
                               trace=_trace)
    out = np.concatenate(
        [res.results[r]["out"].reshape(BPC, T, H) for r in range(NCORES)],
        axis=0)
    if _trace:
        kernel.last_results = res
    return out


# revision 27
# speedup vs baseline: 1.2199x; 1.0062x over previous
"""Trainium2 Bass kernel for nn_Encoder_Postnet (length-regulator gather + per-frame linears).

Contract: kernel(**inputs) takes FULL numpy inputs (as produced by
setup_inputs) and returns the FULL [B, T, H] float32 output. Internally the
batch dim is sharded across 8 NeuronCores (pure data parallel, 4 batches per
core); the tiny Linear(1,H) params are replicated.

Per-core algorithm (BPC=4 batches, T=4096 frames, P=512 phonemes, H=512):
  1. idx[b,t] = cumsum_t(align[b,t] != align[b,t-1])  -- DVE compare + scan
  2. PE-transpose idx chunks into per-partition layout -> gather offsets
  3. grouped indirect-DMA gathers: enc[b, idx[b,t], :] rows from HBM
  4. rank-4 PE matmul per 128-frame tile:
       [pitch_t, beats_t, t, 1] @ [w_pitch; w_beats; w_pos; b_sum]
  5. one DVE add (gathered + psum) per tile, HWDGE write out
"""

import sys

if "/opt/trn_rl_repo" not in sys.path:
    sys.path.insert(0, "/opt/trn_rl_repo")

from contextlib import ExitStack

import numpy as np

import concourse.bass as bass
import concourse.tile as tile
from concourse import bacc, mybir
from concourse.bass_utils import run_bass_kernel_spmd
from concourse.masks import make_identity

B, T, P, H = 32, 4096, 512, 512
NCORES = 8
BPC = B // NCORES            # batches per core
TILE_T = 128                 # frames per tile (partition dim)
NCHUNK = T // TILE_T         # 32 tiles per batch
K_MM = 11                    # bf16 matmul contraction (hi/lo split, see below)
F32 = mybir.dt.float32
BF16 = mybir.dt.bfloat16
I32 = mybir.dt.int32
ADD = mybir.AluOpType.add
SUB = mybir.AluOpType.subtract
NE = mybir.AluOpType.not_equal


def _emit(ctx: ExitStack, tc: tile.TileContext, enc, pitch_bf, beats_bf,
          align, w_pitch, w_beats, w_pos, b_pitch, b_beats, b_pos, aux,
          out):
    nc = tc.nc
    const = ctx.enter_context(tc.tile_pool(name="const", bufs=1))
    apool = ctx.enter_context(tc.tile_pool(name="apool", bufs=1))
    gpool = ctx.enter_context(tc.tile_pool(name="gpool", bufs=10))
    ppool = ctx.enter_context(tc.tile_pool(name="ppool", bufs=6, space="PSUM"))
    tpsum = ctx.enter_context(tc.tile_pool(name="tpsum", bufs=1, space="PSUM"))

    # --- idx chain first: everything else waits on nothing, but the first
    # gather waits on align -> change -> scan -> transpose -> offsets
    align_sb = const.tile([BPC, T], I32)
    nc.sync.dma_start(align_sb[:], align[:])
    change = const.tile([BPC, T], F32)
    nc.vector.memset(change[:, 0:1], 0.0)
    zeros = const.tile([BPC, T], F32)
    nc.vector.memset(zeros[:], 0.0)
    idxf = const.tile([BPC, T], F32)
    ident = const.tile([BPC, BPC], F32)
    make_identity(nc, ident[:])
    idxT_ps = tpsum.tile([TILE_T, NCHUNK * BPC], F32)
    idxT = const.tile([TILE_T, NCHUNK * BPC], F32)
    offs = [const.tile([TILE_T, NCHUNK], I32, tag=f"offs{b}",
                       name=f"offs{b}")
            for b in range(BPC)]
    idxT3 = idxT[:].rearrange("p (c b) -> p b c", b=BPC)  # [128, BPC, NCHUNK]

    NSTAGE = 4
    CPS = NCHUNK // NSTAGE            # chunks per stage
    SW = CPS * TILE_T                 # scan window

    def emit_scan_stage(st):
        lo, hi = st * SW, (st + 1) * SW
        s0 = max(lo, 1)
        nc.vector.tensor_tensor(change[:, s0:hi], align_sb[:, s0:hi],
                                align_sb[:, s0 - 1:hi - 1], op=NE)
        carry = 0.0 if st == 0 else idxf[:, lo - 1:lo]
        nc.vector.tensor_tensor_scan(idxf[:, lo:hi], change[:, lo:hi],
                                     zeros[:, lo:hi], carry,
                                     op0=ADD, op1=ADD)
        for c in range(st * CPS, (st + 1) * CPS):
            nc.tensor.transpose(idxT_ps[:, c * BPC:(c + 1) * BPC],
                                idxf[:, c * TILE_T:(c + 1) * TILE_T],
                                ident[:])
        cl, ch = st * CPS * BPC, (st + 1) * CPS * BPC
        nc.vector.tensor_copy(idxT[:, cl:ch], idxT_ps[:, cl:ch])
        for b in range(BPC):
            nc.vector.tensor_scalar_add(
                offs[b][:, st * CPS:(st + 1) * CPS],
                idxT3[:, b, st * CPS:(st + 1) * CPS], float(b * P))

    emit_scan_stage(0)

    # --- W [11, H] bf16. fp32 matmul lowers to two ~1us passes on TRN2, so
    # the rank-update runs in bf16 with hi/lo-split weights for precision:
    #   pos*w_pos = (t_hi + t_lo) * (w_hi + w_lo),  t_hi = 16*(t//16), exact
    # W rows: [wpos_hi, wpos_lo, wpos_hi, wpos_lo, wpit_hi, wpit_lo,
    #          wbea_hi, wbea_lo, b_pitch, b_beats, b_pos]
    # A rows: [t_hi, t_hi, t_lo, t_lo, pitch, pitch, beats, beats, 1, 1, 1]
    # All row writes land on non-zero partitions via DMA only (compute ops
    # require start partition in {0,32,64,96}).
    W = const.tile([K_MM, H], BF16)
    for i, w in enumerate([w_pos, w_pitch, w_beats]):
        wsrc = const.tile([1, H], F32, tag=f"wsrc{i}")
        nc.sync.dma_start(wsrc[:], w[None, :])
        whi = const.tile([1, H], BF16, tag=f"whi{i}")
        nc.vector.tensor_copy(whi[:], wsrc[:])
        wlo = const.tile([1, H], BF16, tag=f"wlo{i}")
        nc.vector.tensor_tensor(wlo[:], wsrc[:], whi[:], op=SUB)
        if i == 0:  # w_pos used by both t_hi and t_lo rows
            nc.sync.dma_start(W[0:1, :], whi[:])
            nc.sync.dma_start(W[1:2, :], wlo[:])
            nc.sync.dma_start(W[2:3, :], whi[:])
            nc.sync.dma_start(W[3:4, :], wlo[:])
        else:
            nc.sync.dma_start(W[2 + 2 * i:3 + 2 * i, :], whi[:])
            nc.sync.dma_start(W[3 + 2 * i:4 + 2 * i, :], wlo[:])
    nc.gpsimd.dma_start(W[8:9, :], b_pitch[None, :])   # f32 -> bf16 cast DMA
    nc.gpsimd.dma_start(W[9:10, :], b_beats[None, :])
    nc.gpsimd.dma_start(W[10:11, :], b_pos[None, :])

    # --- A tiles, persistent per batch: [t_hi, t_hi, t_lo, t_lo, pitch,
    # pitch, beats, beats, 1, 1, 1]; t_hi/t_lo/ones from host aux and
    # pitch/beats pre-cast to bf16 on the host (exact-layout marshaling)
    As = []
    for b in range(BPC):
        A = apool.tile([K_MM, T], BF16, tag=f"A{b}")
        nc.sync.dma_start(A[0:4, :], aux[0:4, :])
        nc.sync.dma_start(A[4:5, :], pitch_bf[b:b + 1, :])
        nc.sync.dma_start(A[5:6, :], pitch_bf[b:b + 1, :])
        nc.sync.dma_start(A[6:7, :], beats_bf[b:b + 1, :])
        nc.sync.dma_start(A[7:8, :], beats_bf[b:b + 1, :])
        nc.sync.dma_start(A[8:11, :], aux[4:7, :])
        As.append(A)

    for st in range(NSTAGE):
        # emit the NEXT stage's scan chain before this stage's main loop so
        # the DVE resolves stage st+1 offsets while stage st DMAs run
        if st + 1 < NSTAGE:
            emit_scan_stage(st + 1)

        for b in range(BPC):
            for c in range(st * CPS, (st + 1) * CPS):
                # HW indirect DMA consumes exactly one offset per dest
                # partition: per-chunk gathers, 128 descriptors x one H-row
                gt = gpool.tile([TILE_T, H], F32)
                nc.gpsimd.indirect_dma_start(
                    out=gt[:],
                    out_offset=None,
                    in_=enc[:],
                    in_offset=bass.IndirectOffsetOnAxis(
                        ap=offs[b][:, c:c + 1], axis=0),
                )
                ps = ppool.tile([TILE_T, H], F32)
                nc.tensor.matmul(ps[:],
                                 lhsT=As[b][:, c * TILE_T:(c + 1) * TILE_T],
                                 rhs=W[:], start=True, stop=True)
                nc.vector.tensor_tensor(gt[:], gt[:], ps[:], op=ADD)
                # alternate the two HWDGE rings (SP via sync, ACT via scalar)
                weng = nc.sync if c % 2 == 0 else nc.scalar
                weng.dma_start(
                    out[b * T + c * TILE_T: b * T + (c + 1) * TILE_T, :],
                    gt[:])


_CACHED = None


def _build():
    global _CACHED
    if _CACHED is not None:
        return _CACHED
    nc = bacc.Bacc("TRN2", target_bir_lowering=False, debug=False,
                   num_swdge_queues=2)
    enc = nc.dram_tensor("enc", (BPC * P, H), F32, kind="ExternalInput").ap()
    pitch_bf = nc.dram_tensor("pitch_bf", (BPC, T), BF16,
                              kind="ExternalInput").ap()
    beats_bf = nc.dram_tensor("beats_bf", (BPC, T), BF16,
                              kind="ExternalInput").ap()
    align = nc.dram_tensor("align", (BPC, T), I32, kind="ExternalInput").ap()
    w_pitch = nc.dram_tensor("w_pitch", (H,), F32, kind="ExternalInput").ap()
    w_beats = nc.dram_tensor("w_beats", (H,), F32, kind="ExternalInput").ap()
    w_pos = nc.dram_tensor("w_pos", (H,), F32, kind="ExternalInput").ap()
    b_pitch = nc.dram_tensor("b_pitch", (H,), F32, kind="ExternalInput").ap()
    b_beats = nc.dram_tensor("b_beats", (H,), F32, kind="ExternalInput").ap()
    b_pos = nc.dram_tensor("b_pos", (H,), F32, kind="ExternalInput").ap()
    aux = nc.dram_tensor("aux", (7, T), BF16, kind="ExternalInput").ap()
    out = nc.dram_tensor("out", (BPC * T, H), F32, kind="ExternalOutput").ap()

    with tile.TileContext(nc) as tc:
        with ExitStack() as ctx:
            _emit(ctx, tc, enc, pitch_bf, beats_bf, align, w_pitch,
                  w_beats, w_pos, b_pitch, b_beats, b_pos, aux, out)
    nc.compile()
    _CACHED = nc
    return nc


def make_in_maps(encoder_out, pitch, beats, align_phone,
                 w_pitch, b_pitch, w_beats, b_beats, w_pos, b_pos):
    import ml_dtypes
    t = np.arange(T, dtype=np.float32)
    t_hi = np.float32(16.0) * np.floor(t / 16.0).astype(np.float32)
    t_lo = t - t_hi
    ones = np.ones(T, np.float32)
    aux = np.stack([t_hi, t_hi, t_lo, t_lo, ones, ones, ones]).astype(
        ml_dtypes.bfloat16)
    reps = {
        "aux": aux,
        "w_pitch": np.ascontiguousarray(w_pitch, np.float32),
        "w_beats": np.ascontiguousarray(w_beats, np.float32),
        "w_pos": np.ascontiguousarray(w_pos, np.float32),
        "b_pitch": np.ascontiguousarray(b_pitch, np.float32),
        "b_beats": np.ascontiguousarray(b_beats, np.float32),
        "b_pos": np.ascontiguousarray(b_pos, np.float32),
    }
    in_maps = []
    for r in range(NCORES):
        s = slice(r * BPC, (r + 1) * BPC)
        in_maps.append({
            "enc": np.ascontiguousarray(
                encoder_out[s], np.float32).reshape(BPC * P, H),
            "pitch_bf": np.ascontiguousarray(pitch[s]).astype(
                ml_dtypes.bfloat16),
            "beats_bf": np.ascontiguousarray(beats[s]).astype(
                ml_dtypes.bfloat16),
            "align": np.ascontiguousarray(align_phone[s], np.int32),
            **reps,
        })
    return in_maps


def kernel(encoder_out, pitch, beats, w_pitch, b_pitch, w_beats, b_beats,
           w_pos, b_pos, align_phone, _trace=False):
    nc = _build()
    in_maps = make_in_maps(encoder_out, pitch, beats, align_phone,
                           w_pitch, b_pitch, w_beats, b_beats, w_pos, b_pos)
    res = run_bass_kernel_spmd(nc, in_maps, core_ids=list(range(NCORES)),
                               trace=_trace)
    out = np.concatenate(
        [res.results[r]["out"].reshape(BPC, T, H) for r in range(NCORES)],
        axis=0)
    if _trace:
        kernel.last_results = res
    return out
